# revision 1
# baseline (speedup 1.0000x reference)
"""SAGAN-style self-attention block on 8 Trainium2 NeuronCores.

Reference computation (per batch element b, data-parallel over B=8):
    theta = w_theta @ x                         [16, 4096]
    phi   = maxpool2x2(w_phi @ x)               [16, 1024]
    g     = maxpool2x2(w_g @ x)                 [64, 1024]
    scores= theta^T @ phi                       [4096, 1024]
    beta  = softmax_j(scores)
    o     = g @ beta^T                          [64, 4096]
    out   = gamma * (w_o @ o) + x               [128, 4096]

Device algorithm (one batch element per core):
  * theta is never materialized: scores^T = A^T @ x with A = w_theta^T @ phi,
    so the big matmul runs with K=128 instead of K=16.
  * scores^T is kept j-on-partitions; softmax runs without max subtraction
    (max |score| ~ 37 for this distribution, exp stays in fp32 range) and the
    denominator comes from an all-ones row prepended to g^T, so no partition
    reductions or big transposes are needed.
  * normalization is deferred past the output conv: out = (w_o' @ o_unnorm)
    * bcast(1/d) + x, with gamma folded into w_o' on the host and the
    per-column 1/d broadcast done with a K=1 matmul.
"""

import os

import numpy as np

import concourse.bass as bass
import concourse.bacc as bacc
import concourse.tile as tile
from concourse import mybir
from concourse.bass_utils import run_bass_kernel_spmd
from concourse.masks import make_identity

F32 = mybir.dt.float32
F32R = mybir.dt.float32r
BF16 = mybir.dt.bfloat16

C = 128          # channels
HW = 4096        # 64*64 spatial
HWP = 1024       # pooled spatial (32*32)
C8 = 16          # C // 8
C2 = 64          # C // 2
NCORES = 8
NCH = 8          # x is processed in 8 chunks of 512 columns
CHUNK = HW // NCH  # 512

LAST_RESULTS = None
SCORES_BF16 = True


def _emit(nc: bass.Bass, tc: tile.TileContext, x_d, wgp_d, wth_d, wog_d, out_d, pfx=""):
    import contextlib

    with contextlib.ExitStack() as ctx:
        singles = ctx.enter_context(tc.tile_pool(name=pfx + "singles", bufs=1))

        # dummy exp so the ACT function table loads at kernel start
        dummy = singles.tile([1, 1], F32, tag="dummy")
        nc.vector.memset(dummy, 0.0)
        nc.scalar.activation(out=dummy, in_=dummy, func=mybir.ActivationFunctionType.Exp)

        # ---- constants / weights -------------------------------------------------
        wgp_sb = singles.tile([C, C2 + C8], BF16, tag="wgp")     # [128, 80]
        nc.sync.dma_start(out=wgp_sb, in_=wgp_d)
        wth_sb = singles.tile([C2 + C8, C], BF16, tag="wth")      # rows 64:80 used
        wog_sb = singles.tile([C2 + 1, C], BF16, tag="wog")      # [65, 128]
        ident = singles.tile([C, C], F32, tag="ident")


        x_sb = [singles.tile([C, CHUNK], F32, tag=f"x{k}", name=f"{pfx}x{k}") for k in range(NCH)]
        x_bf = [singles.tile([C, CHUNK], BF16, tag=f"xb{k}", name=f"{pfx}xb{k}") for k in range(NCH)]
        pool = singles.tile([C2 + C8, 32, 32], BF16, tag="pool")  # 2x2-pooled [80, 32, 32]
        pool_f = pool.rearrange("p a b -> p (a b)")
        a_sb = singles.tile([C, HWP], BF16, tag="a")             # A = w_theta^T @ phi
        gaug = [singles.tile([C, C2 + 1], BF16, tag=f"gaug{j}", name=f"{pfx}gaug{j}") for j in range(NCH)]
        e_sb = [
            singles.tile([C, HW], BF16, tag=f"e{j}", name=f"{pfx}e{j}")
            for j in range(NCH)
        ]
        r_sb = singles.tile([1, HW], F32, tag="r")

        with tc.tile_pool(name=pfx + "sc_psum", bufs=2, space="PSUM") as sc_psum:
            # ---- pre stage: conv -> 2x2 maxpool -> A chunk -> g^T block ----------
            with tc.tile_pool(name=pfx + "pre_psum", bufs=1, space="PSUM") as pre_psum:
                for k in range(NCH):
                    if k == 0:
                        nc.sync.dma_start(out=wth_sb[C2 : C2 + C8, :], in_=wth_d)
                    nc.sync.dma_start(
                        out=x_sb[k], in_=x_d[:, k * CHUNK : (k + 1) * CHUNK]
                    )
                    nc.gpsimd.dma_start(
                        out=x_bf[k], in_=x_d[:, k * CHUNK : (k + 1) * CHUNK]
                    )
                    if k == 0:
                        make_identity(nc, ident)
                    elif k == 3:
                        nc.sync.dma_start(out=wog_sb, in_=wog_d)
                    ps_gp = pre_psum.tile([C2 + C8, CHUNK], F32, tag="gp", bufs=2)
                    nc.tensor.matmul(ps_gp, wgp_sb, x_bf[k])
                    # fused 2x2 maxpool: [80, (4h, 2hp, 32w, 2wp)] -> [80, 4, 32]
                    v = ps_gp.rearrange(
                        "p (h hp w wp) -> p h w hp wp", h=4, hp=2, w=32, wp=2
                    )
                    nc.vector.tensor_reduce(
                        out=pool[:, k * 4 : (k + 1) * 4, :],
                        in_=v,
                        axis=mybir.AxisListType.XY,
                        op=mybir.AluOpType.max,
                    )
                    # A chunk = w_theta^T @ phi[:, 128 cols] (fp32, rounded on copy)
                    ps_a = pre_psum.tile([C, C], F32, tag="a", bufs=1)
                    nc.tensor.matmul(
                        ps_a,
                        wth_sb[C2 : C2 + C8, :],
                        pool_f[C2 : C2 + C8, k * C : (k + 1) * C],
                    )
                    nc.vector.tensor_copy(out=a_sb[:, k * C : (k + 1) * C], in_=ps_a)

                # g^T blocks via aligned DMA transpose (cols 0:64) + ones col LAST;
                # unaligned transpose destinations corrupt data on HW
                for k in range(NCH):
                    nc.vector.memset(gaug[k][:, C2 : C2 + 1], 1.0)
                    nc.sync.dma_start(
                        out=gaug[k][:, 0:C2],
                        in_=pool_f[0:C2, k * C : (k + 1) * C],
                        transpose=True,
                    )

            # ---- scores^T -> exp ------------------------------------------------
            for icg in range(4):
                for jc in range(NCH):
                    ps_sc = sc_psum.tile([C, 1024], F32, tag="sc")
                    for h in range(2):
                        nc.tensor.matmul(
                            ps_sc[:, h * CHUNK : (h + 1) * CHUNK],
                            a_sb[:, jc * C : (jc + 1) * C],
                            x_bf[icg * 2 + h],
                        )
                    nc.scalar.activation(
                        out=e_sb[jc][:, icg * 1024 : (icg + 1) * 1024],
                        in_=ps_sc,
                        func=mybir.ActivationFunctionType.Exp,
                    )

            # ---- (ones;g)^T E -> normalize -> output conv -> residual -----------
            with tc.tile_pool(name=pfx + "o_psum", bufs=2, space="PSUM") as o_psum:
                for icg in range(4):
                    ps_os = [
                        o_psum.tile([C2 + 1, CHUNK], F32, tag=f"o{h}", name=f"{pfx}o{icg}_{h}")
                        for h in range(2)
                    ]
                    horder = (1, 0) if icg == 3 else (0, 1)
                    for jc in range(NCH):
                        for h in horder:
                            nc.tensor.matmul(
                                ps_os[h],
                                gaug[jc],
                                e_sb[jc][:, (icg * 2 + h) * CHUNK : (icg * 2 + h + 1) * CHUNK],
                                start=(jc == 0),
                                stop=(jc == NCH - 1),
                            )
                    for h in horder:
                        ic = icg * 2 + h
                        ps_o = ps_os[h]
                        rsl = r_sb[0:1, ic * CHUNK : (ic + 1) * CHUNK]
                        nc.vector.reciprocal(out=rsl, in_=ps_o[C2 : C2 + 1, :])
                        r_bc = singles.tile([C2 + 1, CHUNK], F32, tag=f"rbc{ic}", name=f"{pfx}rbc{ic}")
                        nc.gpsimd.partition_broadcast(r_bc, rsl)
                        o_norm = singles.tile([C2 + 1, CHUNK], BF16, tag=f"on{ic}", name=f"{pfx}on{ic}")
                        nc.vector.tensor_mul(o_norm, ps_o, r_bc)
                        ps_oc = o_psum.tile([C, CHUNK], F32, tag=f"o{h}", name=f"{pfx}oc{ic}")
                        t_sb = singles.tile([C, CHUNK], F32, tag=f"t{ic}", name=f"{pfx}t{ic}")
                        if icg < 3 or h == 0:
                            nc.tensor.matmul(ps_oc, wog_sb, o_norm)
                            nc.vector.tensor_add(t_sb, ps_oc, x_sb[ic])
                            out_eng = nc.sync if ic % 2 == 0 else nc.gpsimd
                            out_eng.dma_start(
                                out=out_d[:, ic * CHUNK : (ic + 1) * CHUNK], in_=t_sb
                            )
                        else:
                            # critical last chunk: residual rides the conv psum; ACT evacuates
                            nc.tensor.matmul(
                                ps_oc, ident, x_sb[ic], start=True, stop=False
                            )
                            nc.tensor.matmul(
                                ps_oc, wog_sb, o_norm, start=False, stop=True
                            )
                            for q in range(2):
                                sl = slice(q * (CHUNK // 2), (q + 1) * (CHUNK // 2))
                                nc.scalar.copy(out=t_sb[:, sl], in_=ps_oc[:, sl])
                                out_eng = nc.sync if (ic + q) % 2 == 0 else nc.gpsimd
                                out_eng.dma_start(
                                    out=out_d[:, ic * CHUNK + q * (CHUNK // 2) :
                                              ic * CHUNK + (q + 1) * (CHUNK // 2)],
                                    in_=t_sb[:, sl],
                                )


def _build(nreps=1):
    nc = bacc.Bacc(None)
    x_d = nc.declare_dram_parameter("x", [C, HW], F32, isOutput=False)
    wgp_d = nc.declare_dram_parameter("w_gpT", [C, C2 + C8], BF16, isOutput=False)
    wth_d = nc.declare_dram_parameter("w_th", [C8, C], BF16, isOutput=False)
    wog_d = nc.declare_dram_parameter("w_og", [C2 + 1, C], BF16, isOutput=False)
    out_d = nc.declare_dram_parameter("out", [C, HW], F32, isOutput=True)
    with tile.TileContext(nc) as tc:
        for rep in range(nreps):
            _emit(nc, tc, x_d.ap(), wgp_d.ap(), wth_d.ap(), wog_d.ap(), out_d.ap(),
                  pfx=f"r{rep}_" if nreps > 1 else "")
    nc.compile()
    return nc


_NC = None


def _get_nc():
    global _NC
    if _NC is None:
        _NC = _build()
    return _NC


def _host_weights(w_theta, w_phi, w_g, w_o, gamma):
    w_theta = np.asarray(w_theta, np.float32)
    w_phi = np.asarray(w_phi, np.float32)
    w_g = np.asarray(w_g, np.float32)
    w_o = np.asarray(w_o, np.float32)
    gamma = np.float32(np.asarray(gamma))
    # stationary [128, 80]: columns 0:64 -> g rows, 64:80 -> phi rows
    import ml_dtypes as _mld
    w_gpT = np.ascontiguousarray(np.concatenate([w_g, w_phi], 0).T).astype(_mld.bfloat16)
    w_th = np.ascontiguousarray(w_theta).astype(_mld.bfloat16)
    # [65, 128]: row 0 zero (kills the denominator row), rows 1:65 = (gamma*w_o)^T
    import ml_dtypes
    w_og = np.ascontiguousarray(
        np.concatenate([(gamma * w_o).T, np.zeros((1, C), np.float32)], 0)
    ).astype(ml_dtypes.bfloat16)
    return w_gpT, w_th, w_og


def kernel(inputs, w_theta, w_phi, w_g, w_o, gamma):
    global LAST_RESULTS
    x = np.ascontiguousarray(np.asarray(inputs, np.float32)).reshape(NCORES, C, HW)
    w_gpT, w_th, w_og = _host_weights(w_theta, w_phi, w_g, w_o, gamma)
    nc = _get_nc()
    in_maps = [
        {"x": x[b], "w_gpT": w_gpT, "w_th": w_th, "w_og": w_og}
        for b in range(NCORES)
    ]
    res = run_bass_kernel_spmd(nc, in_maps, list(range(NCORES)))
    LAST_RESULTS = res
    out = np.stack([res.results[b]["out"] for b in range(NCORES)])
    return out.reshape(NCORES, C, 64, 64).astype(np.float32, copy=False)

